# revision 25
# baseline (speedup 1.0000x reference)
"""Trainium2 Bass kernel for nn_CausalMoBEBCNAttention — 8-core SPMD.

Sharding: 8 chunks of 2048 tokens (chunk c = sample c//2, half c%2), one
chunk per NeuronCore.  The causal-cumsum carry into an odd half-chunk is
(sum_t x_even_half) @ MBb by linearity, computed on the host in f32 and
fed as a tiny per-core input — the 8 cores are fully independent (pure
SPMD, no collectives).

All D x D projections are folded on the HOST (f32 numpy) into:
  MBa [D, 2*KR]  x @ MBa = [xV_fwd | xV_inv]        (zA, Q-side)
  MBb [D, 2*KR]  x @ MBb = [yW_fwd | yW_inv]        (zB, K-side, cumsum'd)
  MBr [D, 2*RH]  x @ MBr = router pre-acts (fwd | inv branch)
  CF,CI [KR, D]  expert projection folded with W_O (CI includes alpha)
x is sent pre-transposed ([D, T] bf16) plus a second copy pre-scaled by
the causal 1/n norm (zA side), so the device never transposes x.

Device program (per core, PE ~87% occupied at 2.4 GHz):
  phase A   router h = gelu(xT' @ MBr + b1) and logits = h @ W2T + b2 as
            a 1-step software pipeline over 8 (branch, 512-token-group)
            steps; softmax deferred (exp/copy share one act table: the
            Act engine loads exactly 2 tables for the whole run).
  phase B   per 512-token group, in [kr, t] layout (both matmul operands
            natural — zero transposes anywhere):
              zB = MBb' @ x   -> causal cumsum = DVE tensor_tensor_scan
                                 along t (f32 state; carry = last column
                                 of the previous group's scan)
              zA = MBa' @ xr  -> prod = zA*cum, pw = prod*wexp  (Pool)
              wexp = E-expansion of exp(logits), softmax 1/sum applied
                     via a ones[8,128] matmul broadcast + fast DVE
                     reciprocal; 1/n rides in via the pre-scaled xr
              outT = (CF|CI)' @ pw -> y stored transposed, host flips.
            Group-skewed so the out-projection of group g-1 fills the PE
            while group g's elementwise chains drain on DVE/Pool/Act.

Host keeps a jitted shard_map executable + device-resident folded
weights across calls; per call only xT/xTr (bf16) and the carry move.
`profile_exec()` re-runs the resident executable under the axon NTFF
hook and decodes the per-core profiles with neuron-profile, giving the
true HW execution time (the metric test.py reports).
"""

import sys

if "/opt/trn_rl_repo" not in sys.path:
    sys.path.insert(0, "/opt/trn_rl_repo")

import contextlib
import glob
import json
import os
import subprocess
import tempfile
import time
import types

import numpy as np
import ml_dtypes

import jax
from jax.experimental.shard_map import shard_map
from jax.sharding import Mesh, NamedSharding, PartitionSpec

import concourse.mybir as mybir
import concourse.tile as tile
from concourse import bacc
from concourse.bass2jax import (
    _bass_exec_p,
    install_neuronx_cc_hook,
    partition_id_tensor,
)

F32 = mybir.dt.float32
BF16 = mybir.dt.bfloat16
NPBF = ml_dtypes.bfloat16

B, T, D, R, K = 4, 4096, 1024, 64, 8
RH = 1024
KR = K * R          # 512
KR2 = 2 * KR        # 1024 (fwd+inv)
P = 128
NCORE = 8
TC = T // 2         # 2048 tokens per core
NTC = TC // P       # 16 tiles per core

LAST_EXEC_NS = None
LAST_RUN_WALL_NS = None


# ---------------------------------------------------------------- device


def _build():
    nc = bacc.Bacc("TRN2", target_bir_lowering=False, debug=False, num_devices=1)

    def din(name, shape, dt=BF16):
        return nc.dram_tensor(name, list(shape), dt, kind="ExternalInput")

    xT_d = din("xT", [D, TC])
    xTr_d = din("xTr", [D, TC])                  # x pre-scaled by 1/n (zA side)
    carry_d = din("carry", [P, 8], F32)          # [p, krblock]: kr = blk*128+p
    MBa_d = din("MBa", [D, KR2])
    MBb_d = din("MBb", [D, KR2])
    MBr_d = din("MBr", [D, 2 * RH])
    CF_d = din("CF", [KR, D])
    CI_d = din("CI", [KR, D])
    W2T_d = din("W2T", [RH, K])
    B1_d = din("B1", [P, RH // P], F32)
    B2C_d = din("B2C", [K, 1], F32)
    E_d = din("E", [K, KR])                      # expert-expander (per branch)
    ONES_d = din("ONES8", [K, P])
    y_d = nc.dram_tensor("y", [D, TC], BF16, kind="ExternalOutput")  # yT

    add = mybir.AluOpType.add
    mult = mybir.AluOpType.mult
    byp = mybir.AluOpType.bypass
    ACT = mybir.ActivationFunctionType
    NG = NTC // 4  # 4 groups of 512 tokens

    with tile.TileContext(nc) as tc, contextlib.ExitStack() as top:
        pp = top.enter_context(tc.tile_pool(name="persist", bufs=1))

        def ptile(shape, dt, name):
            return pp.tile(shape, dt, name=name, tag=name)

        # persistent tiles (mbr lives in its own pool, released after phase A)
        xT = ptile([P, 8, TC], BF16, "xT")
        mba = ptile([P, 8, KR2], BF16, "mba")
        mbb = ptile([P, 8, KR2], BF16, "mbb")
        cf = ptile([P, 4, D], BF16, "cf")
        ci = ptile([P, 4, D], BF16, "ci")
        w2t = ptile([P, 8, K], BF16, "w2t")
        b1 = ptile([P, RH // P], F32, "b1")
        b2 = ptile([K, 1], F32, "b2")
        e_sb = ptile([K, KR], BF16, "e_sb")
        ones8 = ptile([K, P], BF16, "ones8")
        xTr = ptile([P, 8, TC], BF16, "xTr")
        carry_sb = ptile([P, 8], F32, "carry_sb")

        smxa = top.enter_context(tc.tile_pool(name="smxa", bufs=8))
        mbrpool = tc.tile_pool(name="mbrpool", bufs=1)
        mbr = mbrpool.__enter__().tile([P, 8, 2 * RH], BF16, name="mbr", tag="mbr")

        def ld(t, sl, dram_ap):
            nc.sync.dma_start(out=t[sl] if sl is not None else t[:],
                              in_=dram_ap)

        def xg(g):
            s = slice(g * 512, (g + 1) * 512)
            ld(xT, (slice(None), slice(None), s),
               xT_d.ap()[:, s].rearrange("(a p) x -> p a x", p=P))

        def mbrp(i):
            s = slice(i * 512, (i + 1) * 512)
            ld(mbr, (slice(None), slice(None), s),
               MBr_d.ap()[:, s].rearrange("(a p) x -> p a x", p=P))

        # first loads split by d-row halves: the first rz matmul only
        # needs kb 0-3 of xT group 0 and the first router columns
        for rows in (slice(0, 512), slice(512, D)):
            a0, a1 = rows.start // P, rows.stop // P
            ld(xT, (slice(None), slice(a0, a1), slice(0, 512)),
               xT_d.ap()[rows, 0:512].rearrange("(a p) x -> p a x", p=P))
            ld(mbr, (slice(None), slice(a0, a1), slice(0, 512)),
               MBr_d.ap()[rows, 0:512].rearrange("(a p) x -> p a x", p=P))
        mbrp(1)
        ld(w2t, None, W2T_d.ap().rearrange("(a p) x -> p a x", p=P))
        ld(b1, None, B1_d.ap())
        ld(b2, None, B2C_d.ap())
        ld(e_sb, None, E_d.ap())
        ld(ones8, None, ONES_d.ap())
        ld(carry_sb, None, carry_d.ap())
        for g in range(1, NG):
            xg(g)
        mbrp(2)
        mbrp(3)
        for hf in range(2):
            s = slice(hf * KR, (hf + 1) * KR)
            ld(mbb, (slice(None), slice(None), s),
               MBb_d.ap()[:, s].rearrange("(a p) x -> p a x", p=P))
        for hf in range(2):
            s = slice(hf * KR, (hf + 1) * KR)
            ld(mba, (slice(None), slice(None), s),
               MBa_d.ap()[:, s].rearrange("(a p) x -> p a x", p=P))
        ld(cf, None, CF_d.ap().rearrange("(a p) x -> p a x", p=P))
        ld(ci, None, CI_d.ap().rearrange("(a p) x -> p a x", p=P))
        for g in range(NG):
            s = slice(g * 512, (g + 1) * 512)
            ld(xTr, (slice(None), slice(None), s),
               xTr_d.ap()[:, s].rearrange("(a p) x -> p a x", p=P))

        # ---- phase A: router h + logits, 1-step software pipeline ----
        steps = [(br, g) for br in range(2) for g in range(NG)]
        lgs_all = []
        with contextlib.ExitStack() as ma:
            rzps = ma.enter_context(tc.tile_pool(name="rzps", bufs=2, space="PSUM"))
            lgps = ma.enter_context(tc.tile_pool(name="lgps", bufs=2, space="PSUM"))
            hpool = ma.enter_context(tc.tile_pool(name="hpool", bufs=2))
            h_ts = {}
            for s in range(len(steps) + 1):
                if s < len(steps):
                    br, g = steps[s]
                    gsl = slice(g * 512, (g + 1) * 512)
                    h_t = hpool.tile([P, 8, 512], BF16, tag="h")
                    h_ts[s] = h_t
                    for rb in range(8):
                        rz = rzps.tile([P, 512], F32, tag="rz")
                        c0 = br * RH + rb * P
                        for kb in range(8):
                            nc.tensor.matmul(
                                rz[:],
                                lhsT=mbr[:, kb, c0:c0 + P],
                                rhs=xT[:, kb, gsl],
                                start=(kb == 0),
                                stop=(kb == 7),
                            )
                        nc.scalar.activation(
                            h_t[:, rb, :], rz[:], ACT.Gelu, bias=b1[:, rb:rb + 1],
                        )
                if s >= 1:
                    h_p = h_ts.pop(s - 1)
                    lg = lgps.tile([K, 512], F32, tag="lg")
                    for rb in range(8):
                        nc.tensor.matmul(
                            lg[:], lhsT=w2t[:, rb, :], rhs=h_p[:, rb, :],
                            start=(rb == 0), stop=(rb == 7),
                        )
                    lgs = smxa.tile([K, 512], F32, tag="lgs")
                    nc.vector.tensor_scalar(lgs[:], lg[:], b2[:, 0:1], None, add)
                    lgs_all.append(lgs)
        mbrpool.__exit__(None, None, None)

        # ---- phase B: [kr, t]-layout expert path per 512-token group ----
        # zB/zA land in PSUM as [kr-block, t]; the causal cumsum is a Pool
        # tensor_tensor_scan along t (f32 state, carry = last column of the
        # previous group''s scan).  The expert weights are expanded to
        # [kr, t] rows by a tiny E-matmul on the exp()''d logits; softmax
        # normalization (1/sum) and the 1/n causal norm ride in as a row
        # factor folded in during the wexp PSUM drain.  The out-projection
        # contracts kr directly (CF/CI already [kr, d]) -> no transposes.
        # No max-subtraction in softmax: logits here are O(1) by
        # construction.
        with contextlib.ExitStack() as mb:
            zps = mb.enter_context(tc.tile_pool(name="zps", bufs=3, space="PSUM"))
            wxps = mb.enter_context(tc.tile_pool(name="wxps", bufs=2, space="PSUM"))
            smps = mb.enter_context(tc.tile_pool(name="smps", bufs=1, space="PSUM"))
            outps = mb.enter_context(tc.tile_pool(name="outps", bufs=2, space="PSUM"))
            cpool = mb.enter_context(tc.tile_pool(name="cpool", bufs=2))
            wxpool = mb.enter_context(tc.tile_pool(name="wxpool", bufs=2))
            pwpool = mb.enter_context(tc.tile_pool(name="pwpool", bufs=2))
            prpool = mb.enter_context(tc.tile_pool(name="prpool", bufs=2))
            ypool = mb.enter_context(tc.tile_pool(name="ypool", bufs=1))
            smxb = mb.enter_context(tc.tile_pool(name="smxb", bufs=2))
            smf = mb.enter_context(tc.tile_pool(name="smf", bufs=1))

            cums = {}   # g -> cumT tile
            wexps = {}  # g -> wexp tile
            pws = {}    # g -> pw tile

            def weights_for(g):
                """softmax + expert-row expansion for both branches of g."""
                wexp = wxpool.tile([P, 2, 4, 512], BF16, tag="wexp")
                wexps[g] = wexp
                for br in range(2):
                    lgs = lgs_all[br * NG + g]
                    ex = smxb.tile([K, 512], BF16, tag="ex")
                    nc.scalar.activation(ex[:], lgs[:], ACT.Exp)
                    # sum of exps broadcast to all 128 partitions in one MM
                    sm = smps.tile([P, 512], F32, tag="sm")
                    nc.tensor.matmul(sm[:], lhsT=ones8[:], rhs=ex[:],
                                     start=True, stop=True)
                    sm_sb = smf.tile([P, 512], F32, tag="smsb")
                    nc.scalar.activation(sm_sb[:], sm[:], ACT.Copy)
                    rcp = smf.tile([P, 512], F32, tag="rcp")
                    nc.vector.reciprocal_approx_fast(out=rcp[:], in_=sm_sb[:])
                    for jb in range(4):
                        wx = wxps.tile([P, 512], F32, tag="wx")
                        nc.tensor.matmul(
                            wx[:], lhsT=e_sb[:, jb * P:(jb + 1) * P], rhs=ex[:],
                            start=True, stop=True)
                        nc.vector.tensor_tensor(
                            wexp[:, br, jb, :], wx[:], rcp[:], mult)

            def group_front(g):
                """zB -> scan -> zA -> prod -> pw for group g."""
                gsl = slice(g * 512, (g + 1) * 512)
                cumT = cpool.tile([P, 8, 512], BF16, tag="cumT")
                cums[g] = cumT
                pw = pwpool.tile([P, 8, 512], BF16, tag="pw")
                pws[g] = pw
                wexp = wexps.pop(g)
                for j in range(8):
                    zB = zps.tile([P, 512], F32, tag="z")
                    for kb in range(8):
                        nc.tensor.matmul(
                            zB[:], lhsT=mbb[:, kb, j * P:(j + 1) * P],
                            rhs=xT[:, kb, gsl],
                            start=(kb == 0), stop=(kb == 7),
                        )
                    if g == 0:
                        init = carry_sb[:, j:j + 1]
                    else:
                        init = cums[g - 1][:, j, 511:512]
                    nc.vector.tensor_tensor_scan(
                        cumT[:, j, :], zB[:], mba[:, 0, 0:512], init, add, byp)
                for j in range(8):
                    br, jb = j // 4, j % 4
                    zA = zps.tile([P, 512], F32, tag="z")
                    for kb in range(8):
                        nc.tensor.matmul(
                            zA[:], lhsT=mba[:, kb, j * P:(j + 1) * P],
                            rhs=xTr[:, kb, gsl],
                            start=(kb == 0), stop=(kb == 7),
                        )
                    zAsb = prpool.tile([P, 512], BF16, tag="zAsb")
                    nc.scalar.activation(zAsb[:], zA[:], ACT.Copy)
                    prod = prpool.tile([P, 512], F32, tag="prod")
                    nc.gpsimd.tensor_tensor(prod[:], zAsb[:], cumT[:, j, :], mult)
                    nc.gpsimd.tensor_tensor(pw[:, j, :], prod[:],
                                            wexp[:, br, jb, :], mult)
                if g >= 2:
                    del cums[g - 2]

            def proj_m(g, pw, y_sb, m):
                gsl = slice(g * 512, (g + 1) * 512)
                out_ps = outps.tile([P, 512], F32, tag="out")
                for br in range(2):
                    Cm = cf if br == 0 else ci
                    for cb in range(4):
                        nc.tensor.matmul(
                            out_ps[:],
                            lhsT=Cm[:, cb, m * P:(m + 1) * P],
                            rhs=pw[:, br * 4 + cb, :],
                            start=(br == 0 and cb == 0),
                            stop=(br == 1 and cb == 3),
                        )
                nc.scalar.activation(y_sb[:, m, :], out_ps[:], ACT.Copy)
                nc.sync.dma_start(out=y_d.ap()[m * P:(m + 1) * P, gsl],
                                  in_=y_sb[:, m, :])

            def group_back(g):
                """outT projection + store for group g."""
                pw = pws.pop(g)
                y_sb = ypool.tile([P, 8, 512], BF16, tag="ysb")
                for m in range(8):
                    proj_m(g, pw, y_sb, m)

            def group_back2(g1, g2):
                """final two groups interleaved by d-block so the last
                projection never waits on its own drain chain."""
                pw1, pw2 = pws.pop(g1), pws.pop(g2)
                ysb1 = ypool.tile([P, 8, 512], BF16, tag="ysb")
                ysb2 = ypool.tile([P, 8, 512], BF16, tag="ysb2")
                for m in range(8):
                    proj_m(g1, pw1, ysb1, m)
                    proj_m(g2, pw2, ysb2, m)

            weights_for(0)
            for g in range(NG):
                group_front(g)
                if g + 1 < NG:
                    weights_for(g + 1)
                if g == NG - 1:
                    group_back2(g - 1, g)
                elif g >= 1:
                    group_back(g - 1)

    nc.compile()
    return nc


# ---------------------------------------------------------------- session


class _Session:
    """Compiled 8-core shard_map executable with device-resident inputs.

    Inputs are global arrays concatenated over cores on axis 0; each core
    sees its slice (exactly the BIR-declared per-core shape)."""

    def __init__(self, nc):
        install_neuronx_cc_hook()
        self.nc = nc

        partition_name = (nc.partition_id_tensor.name
                          if nc.partition_id_tensor else None)
        in_names, out_names, out_avals = [], [], []
        for alloc in nc.m.functions[0].allocations:
            if not isinstance(alloc, mybir.MemoryLocationSet):
                continue
            name = alloc.memorylocations[0].name
            if alloc.kind == "ExternalInput":
                if name != partition_name:
                    in_names.append(name)
            elif alloc.kind == "ExternalOutput":
                out_names.append(name)
                out_avals.append(jax.core.ShapedArray(
                    tuple(alloc.tensor_shape), mybir.dt.np(alloc.dtype)))
        self.param_names = list(in_names)
        self.out_names = list(out_names)
        all_names = in_names + out_names
        if partition_name is not None:
            all_names = all_names + [partition_name]

        def _body(*args):
            operands = list(args)
            if partition_name is not None:
                operands.append(partition_id_tensor())
            outs = _bass_exec_p.bind(
                *operands,
                out_avals=tuple(out_avals),
                in_names=tuple(all_names),
                out_names=tuple(out_names),
                lowering_input_output_aliases=(),
                sim_require_finite=True,
                sim_require_nnan=True,
                nc=nc,
            )
            return tuple(outs)

        devices = jax.devices()[:NCORE]
        assert len(devices) == NCORE, f"need {NCORE} devices, got {len(devices)}"
        self.mesh = Mesh(np.asarray(devices), ("core",))
        spec = PartitionSpec("core")
        n_args = len(in_names) + len(out_names)
        self.jitfn = jax.jit(
            shard_map(
                _body, mesh=self.mesh,
                in_specs=(spec,) * n_args, out_specs=(spec,) * len(out_names),
                check_rep=False,
            ),
            keep_unused=True,
        )
        self.sharding = NamedSharding(self.mesh, spec)
        # outputs are fully written by the program; resident dummies just
        # bind the NEFF output tensors (never donated, reused every call)
        self.zeros = [
            jax.device_put(
                np.zeros((NCORE * a.shape[0],) + tuple(a.shape[1:]), a.dtype),
                self.sharding)
            for a in out_avals
        ]
        self.resident = {}

    def put(self, name, arr_global):
        self.resident[name] = jax.device_put(
            np.ascontiguousarray(arr_global), self.sharding)

    def run(self):
        args = [self.resident[n] for n in self.param_names]
        return self.jitfn(*args, *self.zeros)


# ---------------------------------------------------------------- host side


def _flv(a):
    # (K, D, R) -> [D, K*R], k-major columns
    a = np.asarray(a, np.float32)
    return np.ascontiguousarray(a.transpose(1, 0, 2).reshape(D, KR))


def _fold(inputs):
    f = lambda k: np.asarray(inputs[k], np.float32)
    WQT = np.ascontiguousarray(f("W_Q").T)
    WKT = np.ascontiguousarray(f("W_K").T)
    WIT = np.ascontiguousarray(f("W_inv").T)
    QI = WQT @ WIT
    KI = WKT @ WIT
    r1t = np.ascontiguousarray(f("router_w1").T)
    WOT = np.ascontiguousarray(f("W_O").T)
    alpha = float(np.asarray(inputs["alpha_bi"]))
    MBa = np.concatenate([WQT @ _flv(inputs["V_fwd"]),
                          QI @ _flv(inputs["W_inv_exp"])], axis=1)
    MBb = np.concatenate([WKT @ _flv(inputs["W_fwd"]),
                          KI @ _flv(inputs["V_inv"])], axis=1)
    MBr = np.concatenate([WQT @ r1t, QI @ r1t], axis=1)
    CF = _flv(inputs["U_fwd"]).T @ WOT
    CI = alpha * (_flv(inputs["U_inv"]).T @ WOT)
    bf = lambda a: np.ascontiguousarray(a).astype(NPBF)
    E = np.zeros((K, KR), np.float32)
    for jb in range(4):
        for p in range(P):
            E[2 * jb + (p >= 64), jb * P + p] = 1.0
    shared = {
        "MBa": bf(MBa), "MBb": bf(MBb), "MBr": bf(MBr),
        "CF": bf(CF), "CI": bf(CI),
        "W2T": bf(np.asarray(inputs["router_w2"], np.float32).T),
        "B1": np.ascontiguousarray(
            np.asarray(inputs["router_b1"], np.float32).reshape(RH // P, P).T),
        "B2C": (np.asarray(inputs["router_b2"], np.float32)
                + np.asarray(inputs["expert_bias"], np.float32)).reshape(K, 1),
        "E": bf(E),
        "ONES8": bf(np.ones((K, P), np.float32)),
    }
    return shared, MBb


_WEIGHT_KEYS = (
    "W_Q", "W_K", "W_O", "W_inv", "V_fwd", "W_fwd", "U_fwd", "b_fwd",
    "V_inv", "W_inv_exp", "U_inv", "b_inv", "router_w1", "router_b1",
    "router_w2", "router_b2", "alpha_bi", "expert_bias",
)

_STATE = {"sess": None, "weights": None}


def _get_session():
    if _STATE["sess"] is None:
        _STATE["sess"] = _Session(_build())
    return _STATE["sess"]


def kernel(**inputs) -> np.ndarray:
    global LAST_EXEC_NS, LAST_RUN_WALL_NS
    t_start = time.time()

    x = np.asarray(inputs["x"], np.float32)
    assert x.shape == (B, T, D), x.shape
    for bname in ("b_fwd", "b_inv"):
        if np.abs(np.asarray(inputs[bname])).max() != 0:
            raise NotImplementedError("nonzero expert bias not supported")

    sess = _get_session()

    weights = {k: np.asarray(inputs[k]) for k in _WEIGHT_KEYS}
    w_same = _STATE["weights"] is not None and all(
        np.array_equal(weights[k], _STATE["weights"][k]) for k in _WEIGHT_KEYS)
    if not w_same:
        shared, MBb_f32 = _fold(inputs)
        for name, arr in shared.items():
            sess.put(name, np.concatenate([arr] * NCORE, axis=0))
        _STATE["weights"] = {k: weights[k].copy() for k in _WEIGHT_KEYS}
        _STATE["MBb_f32"] = MBb_f32

    # per-call inputs: transposed x chunks + carry rows
    xc = x.reshape(B, 2, TC, D)
    xT_g = np.ascontiguousarray(
        xc.transpose(0, 1, 3, 2).reshape(NCORE * D, TC)).astype(NPBF)
    # zA-side copy of x pre-scaled by the causal 1/n norm
    recn0 = 1.0 / np.arange(1, TC + 1, dtype=np.float32)
    recn1 = 1.0 / np.arange(TC + 1, 2 * TC + 1, dtype=np.float32)
    xcr = xc * np.stack([recn0, recn1])[None, :, :, None]
    xTr_g = np.ascontiguousarray(
        xcr.transpose(0, 1, 3, 2).reshape(NCORE * D, TC)).astype(NPBF)
    # carry rows in [p, kr-block] layout: carry_sb[p, j] = carry[j*128+p]
    carry_g = np.zeros((NCORE, P, 8), np.float32)
    MBb_f32 = _STATE["MBb_f32"]
    for b in range(B):
        cv = xc[b, 0].sum(axis=0) @ MBb_f32
        carry_g[2 * b + 1] = cv.reshape(8, P).T
    sess.put("xT", xT_g)
    sess.put("xTr", xTr_g)
    sess.put("carry", carry_g.reshape(NCORE * P, 8))

    outs = sess.run()
    yT_g = np.asarray(outs[0])                     # [8*D, TC] bf16 (yT)
    y = (yT_g.astype(np.float32).reshape(NCORE, D, TC)
         .transpose(0, 2, 1).reshape(B, T, D))

    LAST_RUN_WALL_NS = int((time.time() - t_start) * 1e9)
    return y


# ---------------------------------------------------------------- profiling


def _install_ntff_hook():
    """Register the axon NTFF profile hook (the image's antenv lacks
    axon_hooks; inject it and wire the ctypes hook from trn_agent_boot)."""
    try:
        from antenv.axon_hooks import get_axon_ntff_profile_hook
        hook = get_axon_ntff_profile_hook()
        if hook is not None:
            return hook
    except ImportError:
        pass
    import antenv
    from trn_agent_boot.trn_boot import _ntff_profile_via_ctypes

    mod = types.ModuleType("antenv.axon_hooks")
    _h = {}
    mod.set_axon_ntff_profile_hook = lambda h: _h.__setitem__("hook", h)
    mod.get_axon_ntff_profile_hook = lambda: _h.get("hook")
    sys.modules["antenv.axon_hooks"] = mod
    antenv.axon_hooks = mod
    hook = _ntff_profile_via_ctypes("/opt/axon/libaxon_pjrt.so")
    mod.set_axon_ntff_profile_hook(hook)
    return hook


def profile_exec(outdir=None, keep=False):
    """Re-run the resident executable under the NTFF hook; decode each
    core's profile with neuron-profile; return (max_ns, per_core_ns)."""
    global LAST_EXEC_NS
    sess = _STATE["sess"]
    assert sess is not None and "xT" in sess.resident, "call kernel() first"
    hook = _install_ntff_hook()
    if outdir is None:
        outdir = tempfile.mkdtemp(prefix="ntff_")
    os.makedirs(outdir, exist_ok=True)
    with hook(outdir, list(range(NCORE))):
        outs = sess.run()
        jax.block_until_ready(outs)

    ntffs = sorted(glob.glob(os.path.join(outdir, "*.ntff")))
    assert ntffs, f"no NTFF files in {outdir}"
    # pair each ntff with its executable's neff (same filename prefix)
    procs = []
    for nt in ntffs:
        prefix = nt.split("-device")[0]
        neff = prefix + ".neff"
        assert os.path.exists(neff), neff
        js = nt + ".json"
        cmd = ["neuron-profile", "view", "--ignore-nc-buf-usage",
               "-n", neff, "-s", nt, "--output-format=json",
               f"--output-file={js}"]
        procs.append((nt, js, subprocess.Popen(
            cmd, stdout=subprocess.DEVNULL, stderr=subprocess.DEVNULL)))
    per_core = []
    for nt, js, p in procs:
        p.wait()
        assert p.returncode == 0, f"neuron-profile failed on {nt}"
        with open(js) as f:
            summ = json.load(f)["summary"][0]
        per_core.append(int(float(summ["total_time"]) * 1e9))
    LAST_EXEC_NS = max(per_core)
    return LAST_EXEC_NS, per_core


# revision 26
# speedup vs baseline: 1.0024x; 1.0024x over previous
"""Trainium2 Bass kernel for nn_CausalMoBEBCNAttention — 8-core SPMD.

Sharding: 8 chunks of 2048 tokens (chunk c = sample c//2, half c%2), one
chunk per NeuronCore.  The causal-cumsum carry into an odd half-chunk is
(sum_t x_even_half) @ MBb by linearity, computed on the host in f32 and
fed as a tiny per-core input — the 8 cores are fully independent (pure
SPMD, no collectives).

All D x D projections are folded on the HOST (f32 numpy) into:
  MBa [D, 2*KR]  x @ MBa = [xV_fwd | xV_inv]        (zA, Q-side)
  MBb [D, 2*KR]  x @ MBb = [yW_fwd | yW_inv]        (zB, K-side, cumsum'd)
  MBr [D, 2*RH]  x @ MBr = router pre-acts (fwd | inv branch)
  CF,CI [KR, D]  expert projection folded with W_O (CI includes alpha)
x is sent pre-transposed ([D, T] bf16) plus a second copy pre-scaled by
the causal 1/n norm (zA side), so the device never transposes x.

Device program (per core, PE ~87% occupied at 2.4 GHz):
  phase A   router h = gelu(xT' @ MBr + b1) and logits = h @ W2T + b2 as
            a 1-step software pipeline over 8 (branch, 512-token-group)
            steps; softmax deferred (exp/copy share one act table: the
            Act engine loads exactly 2 tables for the whole run).
  phase B   per 512-token group, in [kr, t] layout (both matmul operands
            natural — zero transposes anywhere):
              zB = MBb' @ x   -> causal cumsum = DVE tensor_tensor_scan
                                 along t (f32 state; carry = last column
                                 of the previous group's scan)
              zA = MBa' @ xr  -> prod = zA*cum, pw = prod*wexp  (Pool)
              wexp = E-expansion of exp(logits), softmax 1/sum applied
                     via a ones[8,128] matmul broadcast + fast DVE
                     reciprocal; 1/n rides in via the pre-scaled xr
              outT = (CF|CI)' @ pw -> y stored transposed, host flips.
            Group-skewed so the out-projection of group g-1 fills the PE
            while group g's elementwise chains drain on DVE/Pool/Act.

Host keeps a jitted shard_map executable + device-resident folded
weights across calls; per call only xT/xTr (bf16) and the carry move.
`profile_exec()` re-runs the resident executable under the axon NTFF
hook and decodes the per-core profiles with neuron-profile, giving the
true HW execution time (the metric test.py reports).
"""

import sys

if "/opt/trn_rl_repo" not in sys.path:
    sys.path.insert(0, "/opt/trn_rl_repo")

import contextlib
import glob
import json
import os
import subprocess
import tempfile
import time
import types

import numpy as np
import ml_dtypes

import jax
from jax.experimental.shard_map import shard_map
from jax.sharding import Mesh, NamedSharding, PartitionSpec

import concourse.mybir as mybir
import concourse.tile as tile
from concourse import bacc
from concourse.bass2jax import (
    _bass_exec_p,
    install_neuronx_cc_hook,
    partition_id_tensor,
)

F32 = mybir.dt.float32
BF16 = mybir.dt.bfloat16
NPBF = ml_dtypes.bfloat16

B, T, D, R, K = 4, 4096, 1024, 64, 8
RH = 1024
KR = K * R          # 512
KR2 = 2 * KR        # 1024 (fwd+inv)
P = 128
NCORE = 8
TC = T // 2         # 2048 tokens per core
NTC = TC // P       # 16 tiles per core

LAST_EXEC_NS = None
LAST_RUN_WALL_NS = None


# ---------------------------------------------------------------- device


def _build():
    nc = bacc.Bacc("TRN2", target_bir_lowering=False, debug=False, num_devices=1)

    def din(name, shape, dt=BF16):
        return nc.dram_tensor(name, list(shape), dt, kind="ExternalInput")

    xT_d = din("xT", [D, TC])
    xTr_d = din("xTr", [D, TC])                  # x pre-scaled by 1/n (zA side)
    carry_d = din("carry", [P, 8], F32)          # [p, krblock]: kr = blk*128+p
    MBa_d = din("MBa", [D, KR2])
    MBb_d = din("MBb", [D, KR2])
    MBr_d = din("MBr", [D, 2 * RH])
    CF_d = din("CF", [KR, D])
    CI_d = din("CI", [KR, D])
    W2T_d = din("W2T", [RH, K])
    B1_d = din("B1", [P, RH // P], F32)
    B2C_d = din("B2C", [K, 1], F32)
    E_d = din("E", [K, KR])                      # expert-expander (per branch)
    ONES_d = din("ONES8", [K, P])
    y_d = nc.dram_tensor("y", [D, TC], BF16, kind="ExternalOutput")  # yT

    add = mybir.AluOpType.add
    mult = mybir.AluOpType.mult
    byp = mybir.AluOpType.bypass
    ACT = mybir.ActivationFunctionType
    NG = NTC // 4  # 4 groups of 512 tokens

    with tile.TileContext(nc) as tc, contextlib.ExitStack() as top:
        pp = top.enter_context(tc.tile_pool(name="persist", bufs=1))

        def ptile(shape, dt, name):
            return pp.tile(shape, dt, name=name, tag=name)

        # persistent tiles (mbr lives in its own pool, released after phase A)
        xT = ptile([P, 8, TC], BF16, "xT")
        mba = ptile([P, 8, KR2], BF16, "mba")
        mbb = ptile([P, 8, KR2], BF16, "mbb")
        cf = ptile([P, 4, D], BF16, "cf")
        ci = ptile([P, 4, D], BF16, "ci")
        w2t = ptile([P, 8, K], BF16, "w2t")
        b1 = ptile([P, RH // P], F32, "b1")
        b2 = ptile([K, 1], F32, "b2")
        e_sb = ptile([K, KR], BF16, "e_sb")
        ones8 = ptile([K, P], BF16, "ones8")
        xTr = ptile([P, 8, TC], BF16, "xTr")
        carry_sb = ptile([P, 8], F32, "carry_sb")

        smxa = top.enter_context(tc.tile_pool(name="smxa", bufs=8))
        mbrpool = tc.tile_pool(name="mbrpool", bufs=1)
        mbr = mbrpool.__enter__().tile([P, 8, 2 * RH], BF16, name="mbr", tag="mbr")

        def ld(t, sl, dram_ap):
            nc.sync.dma_start(out=t[sl] if sl is not None else t[:],
                              in_=dram_ap)

        def xg(g):
            s = slice(g * 512, (g + 1) * 512)
            ld(xT, (slice(None), slice(None), s),
               xT_d.ap()[:, s].rearrange("(a p) x -> p a x", p=P))

        def mbrp(i):
            s = slice(i * 512, (i + 1) * 512)
            ld(mbr, (slice(None), slice(None), s),
               MBr_d.ap()[:, s].rearrange("(a p) x -> p a x", p=P))

        # first loads split finely so the first rz matmuls start on
        # ~0.75MB: xT group 0 in kb-pair row slices, first router columns
        # in 256-col pieces (rz consumes mbr columns rb-by-rb)
        ld(mbr, (slice(None), slice(None), slice(0, 256)),
           MBr_d.ap()[:, 0:256].rearrange("(a p) x -> p a x", p=P))
        for a0 in range(0, 8, 2):
            rows = slice(a0 * P, (a0 + 2) * P)
            ld(xT, (slice(None), slice(a0, a0 + 2), slice(0, 512)),
               xT_d.ap()[rows, 0:512].rearrange("(a p) x -> p a x", p=P))
        ld(mbr, (slice(None), slice(None), slice(256, 512)),
           MBr_d.ap()[:, 256:512].rearrange("(a p) x -> p a x", p=P))
        mbrp(1)
        ld(w2t, None, W2T_d.ap().rearrange("(a p) x -> p a x", p=P))
        ld(b1, None, B1_d.ap())
        ld(b2, None, B2C_d.ap())
        ld(e_sb, None, E_d.ap())
        ld(ones8, None, ONES_d.ap())
        ld(carry_sb, None, carry_d.ap())
        for g in range(1, NG):
            xg(g)
        mbrp(2)
        mbrp(3)
        for hf in range(2):
            s = slice(hf * KR, (hf + 1) * KR)
            ld(mbb, (slice(None), slice(None), s),
               MBb_d.ap()[:, s].rearrange("(a p) x -> p a x", p=P))
        for hf in range(2):
            s = slice(hf * KR, (hf + 1) * KR)
            ld(mba, (slice(None), slice(None), s),
               MBa_d.ap()[:, s].rearrange("(a p) x -> p a x", p=P))
        ld(cf, None, CF_d.ap().rearrange("(a p) x -> p a x", p=P))
        ld(ci, None, CI_d.ap().rearrange("(a p) x -> p a x", p=P))
        for g in range(NG):
            s = slice(g * 512, (g + 1) * 512)
            ld(xTr, (slice(None), slice(None), s),
               xTr_d.ap()[:, s].rearrange("(a p) x -> p a x", p=P))

        # ---- phase A: router h + logits, 1-step software pipeline ----
        steps = [(br, g) for br in range(2) for g in range(NG)]
        lgs_all = []
        with contextlib.ExitStack() as ma:
            rzps = ma.enter_context(tc.tile_pool(name="rzps", bufs=2, space="PSUM"))
            lgps = ma.enter_context(tc.tile_pool(name="lgps", bufs=2, space="PSUM"))
            hpool = ma.enter_context(tc.tile_pool(name="hpool", bufs=2))
            h_ts = {}
            for s in range(len(steps) + 1):
                if s < len(steps):
                    br, g = steps[s]
                    gsl = slice(g * 512, (g + 1) * 512)
                    h_t = hpool.tile([P, 8, 512], BF16, tag="h")
                    h_ts[s] = h_t
                    for rb in range(8):
                        rz = rzps.tile([P, 512], F32, tag="rz")
                        c0 = br * RH + rb * P
                        for kb in range(8):
                            nc.tensor.matmul(
                                rz[:],
                                lhsT=mbr[:, kb, c0:c0 + P],
                                rhs=xT[:, kb, gsl],
                                start=(kb == 0),
                                stop=(kb == 7),
                            )
                        nc.scalar.activation(
                            h_t[:, rb, :], rz[:], ACT.Gelu, bias=b1[:, rb:rb + 1],
                        )
                if s >= 1:
                    h_p = h_ts.pop(s - 1)
                    lg = lgps.tile([K, 512], F32, tag="lg")
                    for rb in range(8):
                        nc.tensor.matmul(
                            lg[:], lhsT=w2t[:, rb, :], rhs=h_p[:, rb, :],
                            start=(rb == 0), stop=(rb == 7),
                        )
                    lgs = smxa.tile([K, 512], F32, tag="lgs")
                    nc.vector.tensor_scalar(lgs[:], lg[:], b2[:, 0:1], None, add)
                    lgs_all.append(lgs)
        mbrpool.__exit__(None, None, None)

        # ---- phase B: [kr, t]-layout expert path per 512-token group ----
        # zB/zA land in PSUM as [kr-block, t]; the causal cumsum is a Pool
        # tensor_tensor_scan along t (f32 state, carry = last column of the
        # previous group''s scan).  The expert weights are expanded to
        # [kr, t] rows by a tiny E-matmul on the exp()''d logits; softmax
        # normalization (1/sum) and the 1/n causal norm ride in as a row
        # factor folded in during the wexp PSUM drain.  The out-projection
        # contracts kr directly (CF/CI already [kr, d]) -> no transposes.
        # No max-subtraction in softmax: logits here are O(1) by
        # construction.
        with contextlib.ExitStack() as mb:
            zps = mb.enter_context(tc.tile_pool(name="zps", bufs=3, space="PSUM"))
            wxps = mb.enter_context(tc.tile_pool(name="wxps", bufs=2, space="PSUM"))
            smps = mb.enter_context(tc.tile_pool(name="smps", bufs=1, space="PSUM"))
            outps = mb.enter_context(tc.tile_pool(name="outps", bufs=2, space="PSUM"))
            cpool = mb.enter_context(tc.tile_pool(name="cpool", bufs=2))
            wxpool = mb.enter_context(tc.tile_pool(name="wxpool", bufs=2))
            pwpool = mb.enter_context(tc.tile_pool(name="pwpool", bufs=2))
            prpool = mb.enter_context(tc.tile_pool(name="prpool", bufs=2))
            ypool = mb.enter_context(tc.tile_pool(name="ypool", bufs=1))
            smxb = mb.enter_context(tc.tile_pool(name="smxb", bufs=2))
            smf = mb.enter_context(tc.tile_pool(name="smf", bufs=1))

            cums = {}   # g -> cumT tile
            wexps = {}  # g -> wexp tile
            pws = {}    # g -> pw tile

            def weights_for(g):
                """softmax + expert-row expansion for both branches of g."""
                wexp = wxpool.tile([P, 2, 4, 512], BF16, tag="wexp")
                wexps[g] = wexp
                for br in range(2):
                    lgs = lgs_all[br * NG + g]
                    ex = smxb.tile([K, 512], BF16, tag="ex")
                    nc.scalar.activation(ex[:], lgs[:], ACT.Exp)
                    # sum of exps broadcast to all 128 partitions in one MM
                    sm = smps.tile([P, 512], F32, tag="sm")
                    nc.tensor.matmul(sm[:], lhsT=ones8[:], rhs=ex[:],
                                     start=True, stop=True)
                    sm_sb = smf.tile([P, 512], F32, tag="smsb")
                    nc.vector.tensor_copy(sm_sb[:], sm[:])
                    rcp = smf.tile([P, 512], F32, tag="rcp")
                    nc.vector.reciprocal_approx_fast(out=rcp[:], in_=sm_sb[:])
                    for jb in range(4):
                        wx = wxps.tile([P, 512], F32, tag="wx")
                        nc.tensor.matmul(
                            wx[:], lhsT=e_sb[:, jb * P:(jb + 1) * P], rhs=ex[:],
                            start=True, stop=True)
                        nc.vector.tensor_tensor(
                            wexp[:, br, jb, :], wx[:], rcp[:], mult)

            def group_front(g):
                """zB -> scan -> zA -> prod -> pw for group g."""
                gsl = slice(g * 512, (g + 1) * 512)
                cumT = cpool.tile([P, 8, 512], BF16, tag="cumT")
                cums[g] = cumT
                pw = pwpool.tile([P, 8, 512], BF16, tag="pw")
                pws[g] = pw
                wexp = wexps.pop(g)
                for j in range(8):
                    zB = zps.tile([P, 512], F32, tag="z")
                    for kb in range(8):
                        nc.tensor.matmul(
                            zB[:], lhsT=mbb[:, kb, j * P:(j + 1) * P],
                            rhs=xT[:, kb, gsl],
                            start=(kb == 0), stop=(kb == 7),
                        )
                    if g == 0:
                        init = carry_sb[:, j:j + 1]
                    else:
                        init = cums[g - 1][:, j, 511:512]
                    nc.vector.tensor_tensor_scan(
                        cumT[:, j, :], zB[:], mba[:, 0, 0:512], init, add, byp)
                for j in range(8):
                    br, jb = j // 4, j % 4
                    zA = zps.tile([P, 512], F32, tag="z")
                    for kb in range(8):
                        nc.tensor.matmul(
                            zA[:], lhsT=mba[:, kb, j * P:(j + 1) * P],
                            rhs=xTr[:, kb, gsl],
                            start=(kb == 0), stop=(kb == 7),
                        )
                    zAsb = prpool.tile([P, 512], BF16, tag="zAsb")
                    nc.scalar.activation(zAsb[:], zA[:], ACT.Copy)
                    prod = prpool.tile([P, 512], F32, tag="prod")
                    nc.gpsimd.tensor_tensor(prod[:], zAsb[:], cumT[:, j, :], mult)
                    nc.gpsimd.tensor_tensor(pw[:, j, :], prod[:],
                                            wexp[:, br, jb, :], mult)
                if g >= 2:
                    del cums[g - 2]

            def proj_m(g, pw, y_sb, m):
                gsl = slice(g * 512, (g + 1) * 512)
                out_ps = outps.tile([P, 512], F32, tag="out")
                for br in range(2):
                    Cm = cf if br == 0 else ci
                    for cb in range(4):
                        nc.tensor.matmul(
                            out_ps[:],
                            lhsT=Cm[:, cb, m * P:(m + 1) * P],
                            rhs=pw[:, br * 4 + cb, :],
                            start=(br == 0 and cb == 0),
                            stop=(br == 1 and cb == 3),
                        )
                nc.scalar.activation(y_sb[:, m, :], out_ps[:], ACT.Copy)
                nc.sync.dma_start(out=y_d.ap()[m * P:(m + 1) * P, gsl],
                                  in_=y_sb[:, m, :])

            def group_back(g):
                """outT projection + store for group g."""
                pw = pws.pop(g)
                y_sb = ypool.tile([P, 8, 512], BF16, tag="ysb")
                for m in range(8):
                    proj_m(g, pw, y_sb, m)

            def proj_m_nodma(g, pw, y_sb, m):
                out_ps = outps.tile([P, 512], F32, tag="out")
                for br in range(2):
                    Cm = cf if br == 0 else ci
                    for cb in range(4):
                        nc.tensor.matmul(
                            out_ps[:],
                            lhsT=Cm[:, cb, m * P:(m + 1) * P],
                            rhs=pw[:, br * 4 + cb, :],
                            start=(br == 0 and cb == 0),
                            stop=(br == 1 and cb == 3),
                        )
                nc.scalar.activation(y_sb[:, m, :], out_ps[:], ACT.Copy)

            def group_back2(g1, g2):
                """final two groups interleaved by d-block so the last
                projection never waits on its own drain chain."""
                pw1, pw2 = pws.pop(g1), pws.pop(g2)
                ysb1 = ypool.tile([P, 8, 512], BF16, tag="ysb")
                ysb2 = ypool.tile([P, 8, 512], BF16, tag="ysb2")
                g1sl = slice(g1 * 512, (g1 + 1) * 512)
                for m in range(8):
                    proj_m_nodma(g1, pw1, ysb1, m)
                    proj_m(g2, pw2, ysb2, m)
                nc.sync.dma_start(
                    out=y_d.ap()[:, g1sl].rearrange("(a p) x -> p a x", p=P),
                    in_=ysb1[:])

            weights_for(0)
            for g in range(NG):
                group_front(g)
                if g + 1 < NG:
                    weights_for(g + 1)
                if g == NG - 1:
                    group_back2(g - 1, g)
                elif g >= 1:
                    group_back(g - 1)

    nc.compile()
    return nc


# ---------------------------------------------------------------- session


class _Session:
    """Compiled 8-core shard_map executable with device-resident inputs.

    Inputs are global arrays concatenated over cores on axis 0; each core
    sees its slice (exactly the BIR-declared per-core shape)."""

    def __init__(self, nc):
        install_neuronx_cc_hook()
        self.nc = nc

        partition_name = (nc.partition_id_tensor.name
                          if nc.partition_id_tensor else None)
        in_names, out_names, out_avals = [], [], []
        for alloc in nc.m.functions[0].allocations:
            if not isinstance(alloc, mybir.MemoryLocationSet):
                continue
            name = alloc.memorylocations[0].name
            if alloc.kind == "ExternalInput":
                if name != partition_name:
                    in_names.append(name)
            elif alloc.kind == "ExternalOutput":
                out_names.append(name)
                out_avals.append(jax.core.ShapedArray(
                    tuple(alloc.tensor_shape), mybir.dt.np(alloc.dtype)))
        self.param_names = list(in_names)
        self.out_names = list(out_names)
        all_names = in_names + out_names
        if partition_name is not None:
            all_names = all_names + [partition_name]

        def _body(*args):
            operands = list(args)
            if partition_name is not None:
                operands.append(partition_id_tensor())
            outs = _bass_exec_p.bind(
                *operands,
                out_avals=tuple(out_avals),
                in_names=tuple(all_names),
                out_names=tuple(out_names),
                lowering_input_output_aliases=(),
                sim_require_finite=True,
                sim_require_nnan=True,
                nc=nc,
            )
            return tuple(outs)

        devices = jax.devices()[:NCORE]
        assert len(devices) == NCORE, f"need {NCORE} devices, got {len(devices)}"
        self.mesh = Mesh(np.asarray(devices), ("core",))
        spec = PartitionSpec("core")
        n_args = len(in_names) + len(out_names)
        self.jitfn = jax.jit(
            shard_map(
                _body, mesh=self.mesh,
                in_specs=(spec,) * n_args, out_specs=(spec,) * len(out_names),
                check_rep=False,
            ),
            keep_unused=True,
        )
        self.sharding = NamedSharding(self.mesh, spec)
        # outputs are fully written by the program; resident dummies just
        # bind the NEFF output tensors (never donated, reused every call)
        self.zeros = [
            jax.device_put(
                np.zeros((NCORE * a.shape[0],) + tuple(a.shape[1:]), a.dtype),
                self.sharding)
            for a in out_avals
        ]
        self.resident = {}

    def put(self, name, arr_global):
        self.resident[name] = jax.device_put(
            np.ascontiguousarray(arr_global), self.sharding)

    def run(self):
        args = [self.resident[n] for n in self.param_names]
        return self.jitfn(*args, *self.zeros)


# ---------------------------------------------------------------- host side


def _flv(a):
    # (K, D, R) -> [D, K*R], k-major columns
    a = np.asarray(a, np.float32)
    return np.ascontiguousarray(a.transpose(1, 0, 2).reshape(D, KR))


def _fold(inputs):
    f = lambda k: np.asarray(inputs[k], np.float32)
    WQT = np.ascontiguousarray(f("W_Q").T)
    WKT = np.ascontiguousarray(f("W_K").T)
    WIT = np.ascontiguousarray(f("W_inv").T)
    QI = WQT @ WIT
    KI = WKT @ WIT
    r1t = np.ascontiguousarray(f("router_w1").T)
    WOT = np.ascontiguousarray(f("W_O").T)
    alpha = float(np.asarray(inputs["alpha_bi"]))
    MBa = np.concatenate([WQT @ _flv(inputs["V_fwd"]),
                          QI @ _flv(inputs["W_inv_exp"])], axis=1)
    MBb = np.concatenate([WKT @ _flv(inputs["W_fwd"]),
                          KI @ _flv(inputs["V_inv"])], axis=1)
    MBr = np.concatenate([WQT @ r1t, QI @ r1t], axis=1)
    CF = _flv(inputs["U_fwd"]).T @ WOT
    CI = alpha * (_flv(inputs["U_inv"]).T @ WOT)
    bf = lambda a: np.ascontiguousarray(a).astype(NPBF)
    E = np.zeros((K, KR), np.float32)
    for jb in range(4):
        for p in range(P):
            E[2 * jb + (p >= 64), jb * P + p] = 1.0
    shared = {
        "MBa": bf(MBa), "MBb": bf(MBb), "MBr": bf(MBr),
        "CF": bf(CF), "CI": bf(CI),
        "W2T": bf(np.asarray(inputs["router_w2"], np.float32).T),
        "B1": np.ascontiguousarray(
            np.asarray(inputs["router_b1"], np.float32).reshape(RH // P, P).T),
        "B2C": (np.asarray(inputs["router_b2"], np.float32)
                + np.asarray(inputs["expert_bias"], np.float32)).reshape(K, 1),
        "E": bf(E),
        "ONES8": bf(np.ones((K, P), np.float32)),
    }
    return shared, MBb


_WEIGHT_KEYS = (
    "W_Q", "W_K", "W_O", "W_inv", "V_fwd", "W_fwd", "U_fwd", "b_fwd",
    "V_inv", "W_inv_exp", "U_inv", "b_inv", "router_w1", "router_b1",
    "router_w2", "router_b2", "alpha_bi", "expert_bias",
)

_STATE = {"sess": None, "weights": None}


def _get_session():
    if _STATE["sess"] is None:
        _STATE["sess"] = _Session(_build())
    return _STATE["sess"]


def kernel(**inputs) -> np.ndarray:
    global LAST_EXEC_NS, LAST_RUN_WALL_NS
    t_start = time.time()

    x = np.asarray(inputs["x"], np.float32)
    assert x.shape == (B, T, D), x.shape
    for bname in ("b_fwd", "b_inv"):
        if np.abs(np.asarray(inputs[bname])).max() != 0:
            raise NotImplementedError("nonzero expert bias not supported")

    sess = _get_session()

    weights = {k: np.asarray(inputs[k]) for k in _WEIGHT_KEYS}
    w_same = _STATE["weights"] is not None and all(
        np.array_equal(weights[k], _STATE["weights"][k]) for k in _WEIGHT_KEYS)
    if not w_same:
        shared, MBb_f32 = _fold(inputs)
        for name, arr in shared.items():
            sess.put(name, np.concatenate([arr] * NCORE, axis=0))
        _STATE["weights"] = {k: weights[k].copy() for k in _WEIGHT_KEYS}
        _STATE["MBb_f32"] = MBb_f32

    # per-call inputs: transposed x chunks + carry rows
    xc = x.reshape(B, 2, TC, D)
    xT_g = np.ascontiguousarray(
        xc.transpose(0, 1, 3, 2).reshape(NCORE * D, TC)).astype(NPBF)
    # zA-side copy of x pre-scaled by the causal 1/n norm
    recn0 = 1.0 / np.arange(1, TC + 1, dtype=np.float32)
    recn1 = 1.0 / np.arange(TC + 1, 2 * TC + 1, dtype=np.float32)
    xcr = xc * np.stack([recn0, recn1])[None, :, :, None]
    xTr_g = np.ascontiguousarray(
        xcr.transpose(0, 1, 3, 2).reshape(NCORE * D, TC)).astype(NPBF)
    # carry rows in [p, kr-block] layout: carry_sb[p, j] = carry[j*128+p]
    carry_g = np.zeros((NCORE, P, 8), np.float32)
    MBb_f32 = _STATE["MBb_f32"]
    for b in range(B):
        cv = xc[b, 0].sum(axis=0) @ MBb_f32
        carry_g[2 * b + 1] = cv.reshape(8, P).T
    sess.put("xT", xT_g)
    sess.put("xTr", xTr_g)
    sess.put("carry", carry_g.reshape(NCORE * P, 8))

    outs = sess.run()
    yT_g = np.asarray(outs[0])                     # [8*D, TC] bf16 (yT)
    y = (yT_g.astype(np.float32).reshape(NCORE, D, TC)
         .transpose(0, 2, 1).reshape(B, T, D))

    LAST_RUN_WALL_NS = int((time.time() - t_start) * 1e9)
    return y


# ---------------------------------------------------------------- profiling


def _install_ntff_hook():
    """Register the axon NTFF profile hook (the image's antenv lacks
    axon_hooks; inject it and wire the ctypes hook from trn_agent_boot)."""
    try:
        from antenv.axon_hooks import get_axon_ntff_profile_hook
        hook = get_axon_ntff_profile_hook()
        if hook is not None:
            return hook
    except ImportError:
        pass
    import antenv
    from trn_agent_boot.trn_boot import _ntff_profile_via_ctypes

    mod = types.ModuleType("antenv.axon_hooks")
    _h = {}
    mod.set_axon_ntff_profile_hook = lambda h: _h.__setitem__("hook", h)
    mod.get_axon_ntff_profile_hook = lambda: _h.get("hook")
    sys.modules["antenv.axon_hooks"] = mod
    antenv.axon_hooks = mod
    hook = _ntff_profile_via_ctypes("/opt/axon/libaxon_pjrt.so")
    mod.set_axon_ntff_profile_hook(hook)
    return hook


def profile_exec(outdir=None, keep=False):
    """Re-run the resident executable under the NTFF hook; decode each
    core's profile with neuron-profile; return (max_ns, per_core_ns)."""
    global LAST_EXEC_NS
    sess = _STATE["sess"]
    assert sess is not None and "xT" in sess.resident, "call kernel() first"
    hook = _install_ntff_hook()
    if outdir is None:
        outdir = tempfile.mkdtemp(prefix="ntff_")
    os.makedirs(outdir, exist_ok=True)
    with hook(outdir, list(range(NCORE))):
        outs = sess.run()
        jax.block_until_ready(outs)

    ntffs = sorted(glob.glob(os.path.join(outdir, "*.ntff")))
    assert ntffs, f"no NTFF files in {outdir}"
    # pair each ntff with its executable's neff (same filename prefix)
    procs = []
    for nt in ntffs:
        prefix = nt.split("-device")[0]
        neff = prefix + ".neff"
        assert os.path.exists(neff), neff
        js = nt + ".json"
        cmd = ["neuron-profile", "view", "--ignore-nc-buf-usage",
               "-n", neff, "-s", nt, "--output-format=json",
               f"--output-file={js}"]
        procs.append((nt, js, subprocess.Popen(
            cmd, stdout=subprocess.DEVNULL, stderr=subprocess.DEVNULL)))
    per_core = []
    for nt, js, p in procs:
        p.wait()
        assert p.returncode == 0, f"neuron-profile failed on {nt}"
        with open(js) as f:
            summ = json.load(f)["summary"][0]
        per_core.append(int(float(summ["total_time"]) * 1e9))
    LAST_EXEC_NS = max(per_core)
    return LAST_EXEC_NS, per_core
